# revision 59
# baseline (speedup 1.0000x reference)
"""BertAlibiLayer on 8 TRN2 NeuronCores — data-parallel over batch, v2.4.

Per-core: 2 sequences x 512 tokens, feature-major activations.
Key design vs the 504us v1 baseline:
  - per-sequence software pipelining: QKV(s1)/ctx/Wo PE work hides the
    softmax-exp scalar burst; GLU/Wdown form a second PE-dense superphase;
    12 GLU(s0) chunks are computed early (PSUM->SBUF staged) to fill the
    PE bubble while the last exps drain.
  - fp8e4 DoubleRow matmuls everywhere the error budget allows: QKV, Wo,
    probs@V, GLU, Wdown (2 contraction rows per PE pass). Scores stay
    bf16 (row-packed 2 heads/MM via K=64 tile_position). x ships from the
    host already fp8-packed so the first matmul has no cast dependency.
  - alibi bias is preloaded into PSUM by an identity matmul; the scores
    MM accumulates on top. Scores for a head PAIR land in one 2-bank
    [128,2,512] PSUM tile so a single N=1024 exp (bias=-ln4, fp8 out)
    serves both heads; softmax denominators are also paired (one N=1024
    ln + exp per head pair) off a PSUM-freeing bf16 copy.
  - scalar act-table discipline: exp-set stretch, gelu-set stretch,
    exp-set tail; LN2 is folded into K=1 broadcast matmuls (g2, b2, m,
    rstd) so the output tail needs no scalar at all.

Pool discipline: per-side LIFO stacks — creation order is exactly the
reverse of close order; pools that die early are created last on their
side. wglu is created on top of the wqkv band (freed at P2) so its DMA
starts early.
"""

from contextlib import ExitStack
import os

import numpy as np
import ml_dtypes

# debug bisect flags (read once at program-build time)
USE_RECIP = os.environ.get("K_USE_RECIP", "1") == "1"
USE_WARMUP = os.environ.get("K_USE_WARMUP", "1") == "1"
USE_SCALAR_CT = os.environ.get("K_USE_SCALAR_CT", "0") == "1"
USE_SCALAR_STG = os.environ.get("K_USE_SCALAR_STG", "1") == "1"
# gating multiply (glu_finish stt) on the otherwise-idle gpsimd engine
USE_GPSIMD_STT = os.environ.get("K_USE_GPSIMD_STT", "1") == "1"
# bf16 s1/s2 activations: faster LN stats matmuls + vector ops
USE_BF16_S = os.environ.get("K_USE_BF16_S", "1") == "1"
# default 0: broadcast LN2 mean/rstd via PE matmuls (PE is idle in the
# tail) instead of gpsimd partition_broadcast
USE_LN2_GPSIMD = os.environ.get("K_USE_LN2_GPSIMD", "0") == "1"

import concourse.bass as bass
import concourse.mybir as mybir
import concourse.tile as tile
from concourse import bacc
from concourse.bass_utils import run_bass_kernel_spmd

F32 = mybir.dt.float32
F32R = mybir.dt.float32r
BF16 = mybir.dt.bfloat16
FP8 = mybir.dt.float8e4
AF = mybir.ActivationFunctionType
OP = mybir.AluOpType
DR = mybir.MatmulPerfMode.DoubleRow

DIM = 768
H = 12
HD = 64
S = 512
NSEQ = 2
N = NSEQ * S
I = 3072
KT = DIM // 128   # 6
VW = HD + 1       # 65: ctx copy width (64 head dims + softmax denominator)
VB = 80           # va block stride (16-aligned for dual-fp8 ldweights)
VL = 66           # va loaded width (64 head dims + 2 ones columns, even)
EPS = 1e-12
N_CORES = 8
LN4 = 1.3862943611198906

SX = 64.0   # wqkv / wo scale
SA = 64.0   # wglu gate-half scale
SB = 8.0    # wglu mult-half scale (also the stored gated scale)
SD = 64.0   # wdown scale

NSTAGE = 6   # rolling GLU staging buffers (PSUM-freeing copies)

# set from the actual inputs before the program is built: when ln2_g==1 and
# ln2_b==0 the output tail can skip the per-oc scalar activation entirely
TRIV_LN2 = [False]


def r(ap):
    return ap.bitcast(F32R)


def build_program():
    nc = bacc.Bacc("TRN2", target_bir_lowering=False, debug=False,
                   enable_asserts=False)
    import concourse.hw_specs as hw_specs
    tabs = hw_specs.get_activation_tables(nc.m.arch)
    tabs["natural_log"] = set()

    xT = nc.dram_tensor("xT", [DIM, N], F32, kind="ExternalInput").ap()
    # [p, kp, u, t]: x[(2kp+u)*128+p, t] fp8 (pre-packed moving operand)
    xPK = nc.dram_tensor("xPK", [128, 3, 2, N], FP8, kind="ExternalInput").ap()
    # alibi bias is identical for every sequence -> one copy for both
    btT = nc.dram_tensor("btT", [H, S, S], BF16, kind="ExternalInput").ap()
    ident = nc.dram_tensor("ident", [128, 128], BF16, kind="ExternalInput").ap()
    wqkvPK = nc.dram_tensor("wqkvPK", [128, 3, 2, 3 * DIM], FP8, kind="ExternalInput").ap()
    woPK = nc.dram_tensor("woPK", [128, 3, 2, DIM], FP8, kind="ExternalInput").ap()
    bv_b = nc.dram_tensor("bv_b", [128, DIM], BF16, kind="ExternalInput").ap()
    cpack = nc.dram_tensor("cpack", [128, 84], F32, kind="ExternalInput").ap()
    wgluPK = nc.dram_tensor("wgluPK", [128, 3, 2, 2 * I], FP8, kind="ExternalInput").ap()
    wdownPK = nc.dram_tensor("wdownPK", [128, 6, 2, 2, DIM], FP8, kind="ExternalInput").ap()
    outT = nc.dram_tensor("outT", [DIM, N], F32, kind="ExternalOutput").ap()

    with tile.TileContext(nc) as tc:
        emit(nc, tc, xT, xPK, btT, ident, wqkvPK, woPK, bv_b, cpack,
             wgluPK, wdownPK, outT)

    nc.compile()
    return nc


def emit(nc, tc, xT, xPK, btT, ident, wqkvPK, woPK, bv_b, cpack,
         wgluPK, wdownPK, outT):
    # ---- pool creation, stack-ordered (created first == closed last) ----
    root = ExitStack()
    consts = root.enter_context(tc.tile_pool(name="consts", bufs=1, side="left"))
    pmm = root.enter_context(tc.tile_pool(name="pmm", bufs=2, space="PSUM"))
    pscore = root.enter_context(tc.tile_pool(name="pscore", bufs=2, space="PSUM"))
    pstat = root.enter_context(tc.tile_pool(name="pstat", bufs=2, space="PSUM"))

    t_ctx = ExitStack()
    sq_pool = t_ctx.enter_context(tc.tile_pool(name="sqp", bufs=7, side="left"))
    t_pool = t_ctx.enter_context(tc.tile_pool(name="tts", bufs=2, side="left"))
    zz_pool = t_ctx.enter_context(tc.tile_pool(name="zzs", bufs=2, side="left"))
    st_pool = t_ctx.enter_context(tc.tile_pool(name="stats", bufs=7, side="left"))
    wo_ctx = ExitStack()
    wo_pool = wo_ctx.enter_context(tc.tile_pool(name="wo", bufs=3, side="left"))
    pb_ctx = ExitStack()
    pb_pool = pb_ctx.enter_context(tc.tile_pool(name="pbias", bufs=6, side="left"))
    exp_ctx = ExitStack()
    exp_pool = exp_ctx.enter_context(tc.tile_pool(name="exp", bufs=4, side="left"))
    ctm_pool = exp_ctx.enter_context(tc.tile_pool(name="ctmp", bufs=4, side="left"))
    den_pool = exp_ctx.enter_context(tc.tile_pool(name="den", bufs=2, side="left"))
    bc_pool = exp_ctx.enter_context(tc.tile_pool(name="bcs", bufs=2, side="left"))
    p1_ctx = ExitStack()
    wq_pool = p1_ctx.enter_context(tc.tile_pool(name="wqkv", bufs=3, side="left"))

    rz_ctx = ExitStack()
    r_pool = rz_ctx.enter_context(tc.tile_pool(name="resid", bufs=12, side="right"))
    z1_pool = rz_ctx.enter_context(tc.tile_pool(name="z1pk", bufs=2, side="right"))
    xt1_ctx = ExitStack()
    xt1_pool = xt1_ctx.enter_context(tc.tile_pool(name="xt1", bufs=KT, side="right"))
    ctx_ctx = ExitStack()
    ctx_pool = ctx_ctx.enter_context(tc.tile_pool(name="ctxpk", bufs=2, side="right"))
    s1_ctx = ExitStack()
    s1_pool = s1_ctx.enter_context(tc.tile_pool(name="s1", bufs=12, side="right"))
    qk_ctx = ExitStack()
    qk_pool = qk_ctx.enter_context(tc.tile_pool(name="qk", bufs=12, side="right"))
    va_ctx = ExitStack()
    va_pool = va_ctx.enter_context(tc.tile_pool(name="vaug", bufs=4, side="right"))
    xt0_ctx = ExitStack()
    xt0_pool = xt0_ctx.enter_context(tc.tile_pool(name="xt0", bufs=KT, side="right"))
    xpk_ctx = ExitStack()
    xpk_pool = xpk_ctx.enter_context(tc.tile_pool(name="xpk", bufs=1, side="right"))

    # ---------------- PE warmup -----------------------------------------
    # The HAM clock gate starts cold (1.2 GHz) and only reaches 2.4 GHz
    # after ~3.4us of sustained matmul activity.  The prologue DMAs take
    # ~16us before the first real matmul can run; filling that window with
    # dummy matmuls (no DMA dependency) means P1/P2 run at full clock.
    if USE_WARMUP:
        wt = consts.tile([128, 512], BF16)
        nc.vector.memset(wt[:], 0.125)
        wps = pstat.tile([128, 512], F32, tag="st", name="warm")
        for _ in range(56):
            nc.tensor.matmul(wps[:], wt[:, 0:128], wt[:], start=True,
                             stop=True)

    # ---------------- prologue DMAs --------------------------------------
    xpk = xpk_pool.tile([128, 3, 2, N], FP8, name="xpk", tag="xpk")
    nc.sync.dma_start(xpk[:, :, :, 0:512], xPK[:, :, :, 0:512])
    wq_sb = []
    for kp in range(3):
        t = wq_pool.tile([128, 2, 3 * DIM], FP8, name=f"wq{kp}", tag="wq")
        # split: the first QKV chains need only the q columns — halving the
        # critical prologue bytes starts P1 ~4us earlier
        nc.sync.dma_start(t[:, :, 0:DIM], wqkvPK[:, kp, :, 0:DIM])
        wq_sb.append(t)
    for kp in range(3):
        nc.sync.dma_start(wq_sb[kp][:, :, DIM:3 * DIM],
                          wqkvPK[:, kp, :, DIM:3 * DIM])
    nc.sync.dma_start(xpk[:, :, :, 512:1024], xPK[:, :, :, 512:1024])
    cp_sb = consts.tile([128, 84], F32)
    nc.sync.dma_start(cp_sb[:], cpack[:, :])
    bvb_sb = consts.tile([128, DIM], BF16)
    nc.sync.dma_start(bvb_sb[:], bv_b[:, :])
    ident_sb = consts.tile([128, 128], BF16)
    nc.sync.dma_start(ident_sb[:], ident[:, :])
    xt_sb = [[None] * KT for _ in range(NSEQ)]
    for half in range(2):
        for kt in range(KT):
            pool = xt0_pool if half == 0 else xt1_pool
            t = pool.tile([128, 512], F32, name=f"xt{half}_{kt}",
                          tag=f"xt{half}")
            nc.sync.dma_start(t[:], xT[kt * 128:(kt + 1) * 128,
                                       half * 512:(half + 1) * 512])
            xt_sb[half][kt] = t
    wo_sb = []
    for kp in range(3):
        t = wo_pool.tile([128, 2, DIM], FP8, name=f"wo{kp}", tag="wo")
        nc.sync.dma_start(t[:], woPK[:, kp, :, :])
        wo_sb.append(t)

    bqk_c = cp_sb[:, 0:12]
    cb1_c = cp_sb[:, 12:60]
    g1_c = cp_sb[:, 60:66]
    c1_c = cp_sb[:, 66:72]
    g2_c = cp_sb[:, 72:78]
    b2_c = cp_sb[:, 78:84]

    ones_f32c = consts.tile([128, 12], F32)
    nc.vector.memset(ones_f32c[:], 1.0)
    ones_col = consts.tile([128, 1], F32)
    nc.vector.tensor_copy(ones_col[:].bitcast(F32R), ones_f32c[:, 0:1])
    ones_col_bf = consts.tile([128, 1], BF16)
    nc.vector.memset(ones_col_bf[:], 1.0)
    ones_row = consts.tile([1, 128], F32)
    nc.vector.memset(ones_row[:], 1.0)
    nc.vector.tensor_copy(ones_row[:].bitcast(F32R), ones_row[:])
    eps_sb = consts.tile([1, 1], F32)
    nc.vector.memset(eps_sb[:], EPS)
    nln4_sb = consts.tile([128, 1], F32)
    nc.vector.memset(nln4_sb[:], -LN4)

    bt_sb = {}

    def bt_dma(h):
        bt = pb_pool.tile([128, 4, 512], BF16, name=f"bt{h}", tag="bias",
                          bufs=6)
        nc.gpsimd.dma_start(bt[:], btT[h].rearrange("(c p) i -> p c i", p=128))
        bt_sb[h] = bt



    qk_sb = [[None] * 12 for _ in range(NSEQ)]   # [seq][oc]: oc<6 q, oc>=6 k
    va_sb = [[None] * 2 for _ in range(NSEQ)]    # [seq][scp]: key-chunk pairs
    e_sb = {}                                    # (s, hp) -> [128, 4, 2, 512]
    ctx_sb = [None] * NSEQ                       # packed [128, 3, 2, 512] fp8
    s1_sb = [[None] * KT for _ in range(NSEQ)]
    r_sb = [[None] * KT for _ in range(NSEQ)]
    z1_sb = [z1_pool.tile([128, 3, 2, 512], FP8, name=f"z1_{s}", tag="z1")
             for s in range(NSEQ)]

    # ---------------- helpers -------------------------------------------
    def qkv_chain(s, oc):
        hs = slice(s * 512, (s + 1) * 512)
        ps = pmm.tile([128, 512], F32, tag="ps")
        for kp in range(3):
            nc.tensor.matmul(ps[:], wq_sb[kp][:, :, oc * 128:(oc + 1) * 128],
                             xpk[:, kp, :, hs],
                             start=(kp == 0), stop=(kp == 2), perf_mode=DR)
        t = qk_pool.tile([128, 512], BF16, name=f"qk{s}_{oc}", tag="qk",
                         bufs=12)
        nc.vector.tensor_scalar(t[:], ps[:], 1.0 / SX, bqk_c[:, oc:oc + 1],
                                op0=OP.mult, op1=OP.add)
        qk_sb[s][oc] = t

    def v_chunk(s, sc):
        """one token chunk (128 keys) of the augmented V: 12 blocks of VB.
        Block layout per head: cols 0..1 = ones (denominator rows land at
        PSUM partitions 0/1 so the DVE reciprocal needs no lane shift),
        cols 2..65 = the 64 head dims."""
        scp, u = sc // 2, sc % 2
        if va_sb[s][scp] is None:
            vt = va_pool.tile([128, 2, H * VB], FP8, name=f"va{s}_{scp}",
                              tag="vaug")
            va_sb[s][scp] = vt
            nc.vector.memset(
                vt[:].rearrange("p u (h c) -> p u h c", c=VB)[:, :, :, HD:VL],
                1.0)
        vt_h = va_sb[s][scp][:, u, :].rearrange("p (h c) -> p h c", c=VB)
        col = s * 512 + sc * 128
        for off, width, h0 in ((0, 512, 0), (512, 256, 8)):
            nh = width // HD
            ps = pmm.tile([128, 512], F32, tag="ps")
            for kp in range(3):
                nc.tensor.matmul(ps[:, :width],
                                 xpk[:, kp, :, col:col + 128],
                                 wq_sb[kp][:, :, 2 * DIM + off:2 * DIM + off + width],
                                 start=(kp == 0), stop=(kp == 2), perf_mode=DR)
            nc.vector.scalar_tensor_tensor(
                vt_h[:, h0:h0 + nh, 0:HD],
                ps[:, :width].rearrange("p (h c) -> p h c", c=HD), 1.0 / SX,
                bvb_sb[:, off:off + width].rearrange("p (h c) -> p h c", c=HD),
                op0=OP.mult, op1=OP.add)

    def scores_block(s, hp):
        """scores for head pair (2hp, 2hp+1): per key chunk, both heads'
        scores land in one 2-bank PSUM tile (bias preloaded by identity
        matmuls); a single N=1024 exp emits fp8 probs for both heads."""
        q_t = qk_sb[s][hp]
        k_t = qk_sb[s][6 + hp]
        eh = exp_pool.tile([128, 4, 2, 512], FP8, name=f"e{s}_{hp}",
                           tag="exp", bufs=4)
        e_sb[(s, hp)] = eh
        for jt in range(4):
            js = slice(jt * 128, (jt + 1) * 128)
            ps = pscore.tile([128, 2, 512], F32, tag="sc2")
            for a in range(2):
                h = 2 * hp + a
                rs_ = slice(a * 64, (a + 1) * 64)
                nc.tensor.matmul(ps[:, a, :], ident_sb[:],
                                 bt_sb[h][:, jt, :],
                                 start=True, stop=False)
                nc.tensor.matmul(ps[:, a, :], k_t[rs_, js], q_t[rs_, :],
                                 start=False, stop=True, skip_group_check=True)
            nc.scalar.activation(eh[:, jt, :, :], ps[:], AF.Exp,
                                 bias=nln4_sb[:, 0:1])

    def ctx_pair(s, hp, scalar_ct=False):
        """probs@V for a head pair; denominator reciprocal on the vector
        engine (approx, ~18 bits) off an SBUF-staged copy (the custom DVE
        op cannot read PSUM; only the scalar engine can lane-shift)."""
        eh = e_sb[(s, hp)]
        ct = ctm_pool.tile([HD, 2, 512], BF16, tag="ctmp", bufs=4)
        rc = den_pool.tile([1, 2, 512], F32, tag="rc", bufs=2)
        dd = den_pool.tile([1, 2, 512], F32, tag="dd", bufs=2)
        for a in range(2):
            h = 2 * hp + a
            pc = pmm.tile([128, 512], F32, tag="ps")
            for jtp in range(2):
                nc.tensor.matmul(pc[0:VL, :],
                                 va_sb[s][jtp][:, :, h * VB:h * VB + VL],
                                 eh[:, 2 * jtp:2 * jtp + 2, a, :],
                                 start=(jtp == 0), stop=(jtp == 1),
                                 perf_mode=DR)
            if scalar_ct or USE_SCALAR_CT:
                nc.scalar.copy(ct[:, a, :], pc[0:HD, :])
            else:
                nc.vector.tensor_copy(ct[:, a, :], pc[0:HD, :])
            # den (partition 64 of PSUM) -> SBUF partition 0 via scalar (the
            # only engine that can lane-shift), then DVE approx reciprocal —
            # the custom DVE op cannot read PSUM.
            nc.scalar.copy(dd[0:1, a, :], pc[HD:HD + 1, :])
            nc.vector.reciprocal_approx_fast(rc[0:1, a, :], dd[0:1, a, :])
        if ctx_sb[s] is None:
            ctx_sb[s] = ctx_pool.tile([128, 3, 2, 512], FP8,
                                      name=f"ctx{s}", tag="ctx", bufs=2)
        kt = hp  # feature chunk kt holds heads 2hp, 2hp+1
        for a in range(2):
            bc = bc_pool.tile([64, 512], F32, tag="bc", bufs=2)
            nc.gpsimd.partition_broadcast(bc[:], rc[0:1, a, :], channels=64)
            nc.vector.tensor_mul(
                ctx_sb[s][a * 64:a * 64 + 64, kt // 2, kt % 2, :],
                ct[0:HD, a, :], bc[:])

    S_DT = BF16 if USE_BF16_S else F32

    def wo_chain(s, oc):
        # bo is folded into xT on the host, so no bias preload matmul.
        ps = pmm.tile([128, 512], F32, tag="ps")
        for kp in range(3):
            nc.tensor.matmul(ps[:], wo_sb[kp][:, :, oc * 128:(oc + 1) * 128],
                             ctx_sb[s][:, kp, :, :],
                             start=(kp == 0), stop=(kp == 2), perf_mode=DR)
        t = s1_pool.tile([128, 512], S_DT, name=f"s1_{s}_{oc}", tag="s1",
                         bufs=12)
        dst = t[:] if USE_BF16_S else t[:].bitcast(F32R)
        nc.vector.scalar_tensor_tensor(
            dst, ps[:], 1.0 / SX, xt_sb[s][oc][:],
            op0=OP.mult, op1=OP.add)
        s1_sb[s][oc] = t

    def ln_stats(src_sb, sqs_pre=None):
        psx_t = pstat.tile([1, 512], F32, tag="st", name="psx")
        psxx_t = pstat.tile([1, 512], F32, tag="st", name="psxx")
        for oc in range(KT):
            if sqs_pre is not None:
                sq = sqs_pre[oc]
            else:
                sq = sq_pool.tile([128, 512], BF16, tag="sq", bufs=7)
                nc.vector.tensor_mul(sq[:], src_sb[oc][:], src_sb[oc][:])
            if USE_BF16_S:
                nc.tensor.matmul(psx_t[:], ones_col_bf[:], src_sb[oc][:],
                                 start=(oc == 0), stop=(oc == KT - 1))
            else:
                nc.tensor.matmul(psx_t[:], r(ones_col[:]), r(src_sb[oc][:]),
                                 start=(oc == 0), stop=(oc == KT - 1))
            nc.tensor.matmul(psxx_t[:], ones_col_bf[:], sq[:],
                             start=(oc == 0), stop=(oc == KT - 1))
        m_sb = st_pool.tile([1, 512], F32, tag="st", bufs=7)
        nc.scalar.activation(m_sb[:], psx_t[:], AF.Identity, scale=1.0 / DIM)
        msq = st_pool.tile([1, 512], F32, tag="st", bufs=7)
        nc.scalar.activation(msq[:], psx_t[:], AF.Square, scale=1.0 / DIM)
        var = st_pool.tile([1, 512], F32, tag="st", bufs=7)
        nc.vector.scalar_tensor_tensor(var[:], psxx_t[:], 1.0 / DIM, msq[:],
                                       op0=OP.mult, op1=OP.subtract)
        lv = st_pool.tile([1, 512], F32, tag="st", bufs=7)
        nc.scalar.activation(lv[:], var[:], AF.Ln, bias=eps_sb[:1, :1])
        rs_ = st_pool.tile([1, 512], F32, tag="st", bufs=7)
        nc.scalar.activation(rs_[:], lv[:], AF.Exp, scale=-0.5)
        return m_sb, rs_

    def layernorm1(s, sqs_pre=None):
        m_sb, rs_ = ln_stats(s1_sb[s], sqs_pre)
        m_r = st_pool.tile([1, 512], F32, tag="st", bufs=7)
        nc.vector.tensor_copy(m_r[:].bitcast(F32R), m_sb[:])
        rs_r = st_pool.tile([1, 512], F32, tag="st", bufs=7)
        nc.vector.tensor_copy(rs_r[:].bitcast(F32R), rs_[:])
        mbc = pmm.tile([128, 512], F32, tag="ps")
        nc.tensor.matmul(mbc[:], r(ones_row[:]), r(m_r[:]), start=True, stop=True)
        rbc = pmm.tile([128, 512], F32, tag="ps")
        nc.tensor.matmul(rbc[:], r(ones_row[:]), r(rs_r[:]), start=True, stop=True)
        for oc in range(KT):
            t = t_pool.tile([128, 512], F32, tag="t", bufs=2)
            nc.vector.tensor_sub(t[:], s1_sb[s][oc][:], mbc[:])
            zz = zz_pool.tile([128, 512], F32, tag="zz", bufs=2)
            nc.vector.tensor_mul(zz[:], t[:], rbc[:])
            nc.vector.tensor_copy(z1_sb[s][:, oc // 2, oc % 2, :], zz[:])
            rt = r_pool.tile([128, 512], BF16, name=f"r{s}_{oc}", tag="resid")
            nc.scalar.activation(rt[:], zz[:], AF.Identity,
                                 bias=c1_c[:, oc:oc + 1],
                                 scale=g1_c[:, oc:oc + 1])
            r_sb[s][oc] = rt

    # ---------------- P1: QKV(s0) + V(both seqs) --------------------------
    # bias DMAs issue only after the QKV chains are emitted so the bias
    # stream doesn't steal DMA bandwidth from the prologue transfers that
    # gate the first matmul
    for oc in range(12):
        qkv_chain(0, oc)
    for h in range(4):
        bt_dma(h)
    for sc in range(4):
        v_chunk(0, sc)
        v_chunk(1, sc)

    # ---------------- P2: fused attention superphase ----------------------
    # Each iteration: scores(s0,hp) + q/k(s1) + scores(s1,hp-1) (same bias
    # tiles, loaded once) + both sequences' ctx pairs trailing behind the
    # exps.  One bias copy serves both sequences and the per-head tile is
    # consumed within two iterations.
    for hp in range(6):
        if 2 * hp + 4 < H:
            bt_dma(2 * hp + 4)
            bt_dma(2 * hp + 5)
        scores_block(0, hp)
        qkv_chain(1, hp)
        qkv_chain(1, 6 + hp)
        if hp >= 1:
            scores_block(1, hp - 1)
            ctx_pair(0, hp - 1)
        if hp >= 2:
            ctx_pair(1, hp - 2)
    scores_block(1, 5)
    ctx_pair(0, 5, scalar_ct=True)
    xpk_ctx.close()
    p1_ctx.close()
    wo_chain(0, 0)
    wo_chain(0, 1)
    ctx_pair(1, 4, scalar_ct=True)
    wo_chain(0, 2)
    wo_chain(0, 3)
    ctx_pair(1, 5, scalar_ct=True)
    wo_chain(0, 4)
    wo_chain(0, 5)
    xt0_ctx.close()

    # attention-side frees (before the big MLP pools open)
    va_ctx.close()
    qk_ctx.close()
    exp_ctx.close()
    pb_ctx.close()

    # MLP pools: wd+gated+staging first, wglu on top (lands in the wqkv
    # band, freed at P2 -> its DMA starts early)
    wd_ctx = ExitStack()
    wd_pool = wd_ctx.enter_context(tc.tile_pool(name="wdown", bufs=6, side="left"))
    gated_ctx = ExitStack()
    gated_pool = gated_ctx.enter_context(
        tc.tile_pool(name="gated", bufs=2, side="left"))
    gs_ctx = ExitStack()
    gs_pool = gs_ctx.enter_context(
        tc.tile_pool(name="gstage", bufs=NSTAGE, side="left"))
    wg_ctx = ExitStack()
    wg_pool = wg_ctx.enter_context(tc.tile_pool(name="wglu", bufs=3, side="left"))
    wg_sb = []
    for kp in range(3):
        t = wg_pool.tile([128, 2, 2 * I], FP8, name=f"wg{kp}", tag="wg")
        nc.sync.dma_start(t[:], wgluPK[:, kp, :, :])
        wg_sb.append(t)
    wd_sb = []
    for kq in range(6):
        t = wd_pool.tile([128, 2, 2, DIM], FP8, name=f"wd{kq}", tag="wd")
        nc.sync.dma_start(t[:], wdownPK[:, kq, :, :, :])
        wd_sb.append(t)
    gated_sb = [gated_pool.tile([128, 6, 2, 2, 512], FP8, name=f"gated{s}",
                                tag="gated") for s in range(NSEQ)]

    def glu_chains(s, j):
        """ps pair tile: [:,0,:] gate chain, [:,1,:] mult chain."""
        psp = pscore.tile([128, 2, 512], F32, tag="sc2")
        for kp in range(3):
            nc.tensor.matmul(psp[:, 0, :],
                             wg_sb[kp][:, :, j * 128:(j + 1) * 128],
                             z1_sb[s][:, kp, :, :],
                             start=(kp == 0), stop=(kp == 2), perf_mode=DR)
        for kp in range(3):
            nc.tensor.matmul(psp[:, 1, :],
                             wg_sb[kp][:, :, I + j * 128:I + (j + 1) * 128],
                             z1_sb[s][:, kp, :, :],
                             start=(kp == 0), stop=(kp == 2), perf_mode=DR)
        return psp

    # ---------------- P7/P8: GLU pipeline --------------------------------
    # Every chunk: PE chain -> PSUM-freeing copy (alternating scalar/vector)
    # -> gelu (scalar) || gating stt (gpsimd, otherwise idle all MLP).
    # The rolling stg buffer decouples the PE chain rate from the
    # gelu/gating consumers; the first 12 s0 chunks run under LN1(s1).
    ge_ctx = ExitStack()
    ge_pool = ge_ctx.enter_context(tc.tile_pool(name="gelu", bufs=4, side="left"))

    def glu_chunk(s, j, mul_vector=False):
        # staging copy adds the mult-half bias to BOTH halves (per-partition
        # bias col); the gelu bias is host-adjusted to compensate on the
        # gate half.  The gating multiply is then a plain tensor_tensor that
        # the (otherwise idle) gpsimd engine supports.
        psp = glu_chains(s, j)
        stg = gs_pool.tile([128, 2, 512], BF16, name=f"gst{s}_{j}",
                           tag="gst", bufs=NSTAGE)
        if j % 3 == 0:
            nc.scalar.activation(stg[:], psp[:], AF.Identity,
                                 bias=cb1_c[:, 24 + j:25 + j])
        else:
            nc.vector.tensor_scalar(stg[:], psp[:], 1.0,
                                    cb1_c[:, 24 + j:25 + j],
                                    op0=OP.mult, op1=OP.add)
        ge = ge_pool.tile([128, 512], F32, tag="gelu")
        nc.scalar.activation(ge[:], stg[:, 0, :], AF.Gelu,
                             bias=cb1_c[:, j:j + 1], scale=1.0 / SA)
        dst = gated_sb[s][:, j // 4, (j // 2) % 2, j % 2, :]
        if USE_GPSIMD_STT and not mul_vector:
            nc.gpsimd.tensor_mul(dst, stg[:, 1, :], ge[:])
        else:
            nc.vector.tensor_mul(dst, stg[:, 1, :], ge[:])

    # Wo(s1) + sq first: fills the PE while the LN1(s0) vector chain runs.
    sqs_pre = []
    for oc in range(KT):
        wo_chain(1, oc)
        sq = sq_pool.tile([128, 512], BF16, tag="sq", bufs=7)
        nc.vector.tensor_mul(sq[:], s1_sb[1][oc][:], s1_sb[1][oc][:])
        sqs_pre.append(sq)
    layernorm1(0)
    for j in range(8):
        glu_chunk(0, j)
    layernorm1(1, sqs_pre)

    s1_ctx.close()
    ctx_ctx.close()
    xt1_ctx.close()

    for j in range(8, 24):
        glu_chunk(0, j)
    # the last s1 gating mults go to the vector engine: gpsimd otherwise
    # paces the completion of gated(s1) and delays the Wdown(s1) chains
    for j in range(24):
        glu_chunk(1, j, mul_vector=(j >= 18))
    ge_ctx.close()

    # ---------------- P9/P10: Wdown + LN2 + out ---------------------------
    s2_ctx = ExitStack()
    s2_pool = s2_ctx.enter_context(tc.tile_pool(name="s2", bufs=6, side="left"))
    lnb_pool = s2_ctx.enter_context(tc.tile_pool(name="lnb", bufs=4, side="left"))
    out_ctx = ExitStack()
    out_pool = out_ctx.enter_context(tc.tile_pool(name="outp", bufs=4, side="right"))

    for s in range(NSEQ):
        s2_sb = []
        for oc in range(KT):
            ps_t = pmm.tile([128, 512], F32, tag="ps")
            ps = ps_t[:]
            for kq in range(6):
                for i2 in range(2):
                    nc.tensor.matmul(
                        ps, wd_sb[kq][:, i2, :, oc * 128:(oc + 1) * 128],
                        gated_sb[s][:, kq, i2, :, :],
                        start=(kq == 0 and i2 == 0),
                        stop=(kq == 5 and i2 == 1), perf_mode=DR)
            t = s2_pool.tile([128, 512], S_DT, name=f"s2_{s}_{oc}", tag="s2",
                             bufs=6)
            dst = t[:] if USE_BF16_S else t[:].bitcast(F32R)
            nc.vector.scalar_tensor_tensor(dst, ps,
                                           1.0 / (SB * SD), r_sb[s][oc][:],
                                           op0=OP.mult, op1=OP.add)
            s2_sb.append(t)
        # LN2: mean/rstd broadcast on gpsimd; g2/b2 as per-partition
        # scale/bias in a scalar Identity (no PE broadcast matmuls).
        m_sb, rs_ = ln_stats(s2_sb)
        if USE_LN2_GPSIMD:
            mb = lnb_pool.tile([128, 512], F32, tag="lnb", bufs=4)
            nc.gpsimd.partition_broadcast(mb[:], m_sb[:], channels=128)
            rb = lnb_pool.tile([128, 512], F32, tag="lnb", bufs=4)
            nc.gpsimd.partition_broadcast(rb[:], rs_[:], channels=128)
        else:
            m_r = st_pool.tile([1, 512], F32, tag="st", bufs=7)
            nc.vector.tensor_copy(m_r[:].bitcast(F32R), m_sb[:])
            rs_r = st_pool.tile([1, 512], F32, tag="st", bufs=7)
            nc.vector.tensor_copy(rs_r[:].bitcast(F32R), rs_[:])
            mb = pstat.tile([128, 512], F32, tag="st", name="mbb")
            nc.tensor.matmul(mb[:], r(ones_row[:]), r(m_r[:]),
                             start=True, stop=True)
            rb = pstat.tile([128, 512], F32, tag="st", name="rbb")
            nc.tensor.matmul(rb[:], r(ones_row[:]), r(rs_r[:]),
                             start=True, stop=True)
        for oc in range(KT):
            t1 = t_pool.tile([128, 512], F32, tag="t", bufs=2)
            nc.vector.tensor_sub(t1[:], s2_sb[oc][:], mb[:])
            ot = out_pool.tile([128, 512], F32, tag="out")
            if TRIV_LN2[0]:
                # ln2_g == 1, ln2_b == 0: the scale-mul IS the output
                nc.vector.tensor_mul(ot[:], t1[:], rb[:])
            else:
                t2 = zz_pool.tile([128, 512], F32, tag="zz", bufs=2)
                nc.vector.tensor_mul(t2[:], t1[:], rb[:])
                nc.scalar.activation(ot[:], t2[:], AF.Identity,
                                     bias=b2_c[:, oc:oc + 1],
                                     scale=g2_c[:, oc:oc + 1])
            nc.sync.dma_start(outT[oc * 128:(oc + 1) * 128,
                                   s * 512:(s + 1) * 512], ot[:])

    out_ctx.close()
    s2_ctx.close()
    wg_ctx.close()
    gs_ctx.close()
    gated_ctx.close()
    wd_ctx.close()
    rz_ctx.close()
    wo_ctx.close()
    t_ctx.close()
    root.close()


# ---------------------------------------------------------------------------
_NC_CACHE = None


def _get_nc():
    global _NC_CACHE
    if _NC_CACHE is None:
        _NC_CACHE = build_program()
    return _NC_CACHE


def _to128(v, cols):
    return np.ascontiguousarray(np.asarray(v, np.float32).reshape(cols, 128).T)


def _pack_k(W, scale):
    """W: (out, 768) -> [p, kp, u, out] fp8, k = (2kp+u)*128+p."""
    return np.ascontiguousarray(
        (W * scale).astype(np.float32).T.reshape(3, 2, 128, W.shape[0])
        .transpose(2, 0, 1, 3)).astype(ml_dtypes.float8_e4m3)


def prep_inputs(inputs):
    hs = np.asarray(inputs["hidden_states"], np.float32)
    bias = np.asarray(inputs["bias"], np.float32)
    Wqkv = np.asarray(inputs["Wqkv"], np.float32)
    bqkv = np.asarray(inputs["bqkv"], np.float32)
    Wo = np.asarray(inputs["Wo"], np.float32)
    bo_v = np.asarray(inputs["bo"], np.float32)
    ln1_g = np.asarray(inputs["ln1_g"], np.float32)
    ln1_b = np.asarray(inputs["ln1_b"], np.float32)
    Wglu = np.asarray(inputs["Wglu"], np.float32)
    Wdown = np.asarray(inputs["Wdown"], np.float32)
    bdown = np.asarray(inputs["bdown"], np.float32)
    ln2_g = np.asarray(inputs["ln2_g"], np.float32)
    ln2_b = np.asarray(inputs["ln2_b"], np.float32)

    x_t = np.ascontiguousarray(hs.T)                       # (768, 8192)
    xPK_full = np.ascontiguousarray(
        x_t.reshape(3, 2, 128, 8192).transpose(2, 0, 1, 3)
    ).astype(ml_dtypes.float8_e4m3)                        # [128, 3, 2, 8192]
    # fold the Wo bias into the residual stream: s1 = ctx@Wo/SX + (x + bo)
    x_tb = x_t + bo_v[:, None]
    btT = np.ascontiguousarray(bias.transpose(0, 1, 3, 2)).astype(
        ml_dtypes.bfloat16)
    ident = np.eye(128, dtype=ml_dtypes.bfloat16)

    scale = 1.0 / np.sqrt(np.float32(HD))
    Wq = Wqkv.copy()
    Wq[:DIM] *= scale
    wqkvPK = _pack_k(Wq, SX)
    bqk_v = bqkv[:2 * DIM].copy()
    bqk_v[:DIM] *= scale
    bv_b = np.ascontiguousarray(
        np.broadcast_to(bqkv[2 * DIM:], (128, DIM)).astype(ml_dtypes.bfloat16))
    woPK = _pack_k(Wo, SX)

    Wg_s = (Wglu * ln1_g[None, :]).astype(np.float32)      # (6144, 768)
    Wg_s[:I] *= SA
    Wg_s[I:] *= SB
    wgluPK = np.ascontiguousarray(
        Wg_s.T.reshape(3, 2, 128, 2 * I).transpose(2, 0, 1, 3)
    ).astype(ml_dtypes.float8_e4m3)
    cb1_v = (Wglu @ ln1_b).astype(np.float32)               # (6144,)
    cb1_v[I:] *= SB
    # the staging copy adds the mult-half bias to both halves of the GLU
    # chunk; compensate on the gate half (gelu input is scaled by 1/SA)
    cb1_v[:I] -= cb1_v[I:] / SA
    c1_v = ln1_b + bdown
    Wd_s = (Wdown * SD).astype(np.float32)                  # (768, 3072)
    wdownPK = np.ascontiguousarray(
        Wd_s.T.reshape(6, 2, 2, 128, DIM).transpose(3, 0, 1, 2, 4)
    ).astype(ml_dtypes.float8_e4m3)

    cpack = np.concatenate([
        _to128(bqk_v, 12), _to128(cb1_v, 48), _to128(ln1_g, 6),
        _to128(c1_v, 6), _to128(ln2_g, 6), _to128(ln2_b, 6)],
        axis=1)                                             # [128, 84]

    shared = {
        "ident": ident,
        "wqkvPK": wqkvPK,
        "woPK": woPK,
        "bv_b": bv_b,
        "cpack": np.ascontiguousarray(cpack),
        "wgluPK": wgluPK,
        "wdownPK": wdownPK,
    }
    in_maps = []
    for c in range(N_CORES):
        m = dict(shared)
        m["xT"] = np.ascontiguousarray(x_tb[:, c * N:(c + 1) * N])
        m["xPK"] = np.ascontiguousarray(xPK_full[:, :, :, c * N:(c + 1) * N])
        # alibi bias is broadcast over the batch dim: one copy per core
        m["btT"] = np.ascontiguousarray(btT[c * NSEQ])
        in_maps.append(m)
    return in_maps


def kernel(**inputs):
    global _NC_CACHE
    triv = bool(np.all(np.asarray(inputs["ln2_g"]) == 1.0)
                and np.all(np.asarray(inputs["ln2_b"]) == 0.0))
    if triv != TRIV_LN2[0]:
        TRIV_LN2[0] = triv
        _NC_CACHE = None
    nc = _get_nc()
    in_maps = prep_inputs(inputs)
    res = run_bass_kernel_spmd(nc, in_maps, core_ids=list(range(N_CORES)))
    outT = np.concatenate([res.results[c]["outT"] for c in range(N_CORES)],
                          axis=1)
    return np.ascontiguousarray(outT.T)

